# revision 8
# baseline (speedup 1.0000x reference)
"""Llama trunk (2 layers, before final norm) on 8 trn2 cores.

Sharding: Megatron tensor-parallel over 8 cores.
  - attention: 4 q-heads + 1 kv-head per core (GQA group stays local)
  - MLP: 1024 of 8192 intermediate dims per core
  - residual stream x replicated in f32; matmul operands bf16
Layout: transposed on device: xT is [DM(part), S(free)] so every matmul
contracts over the partition axis.  Final layer's down-proj partials are
reduced on the host (saves one AllReduce); the other 3 AllReduces run
on-device through DRAM bounce buffers.  RMSNorm gains and 1/sqrt(D) are
folded into the weights on the host.
"""
import math
from contextlib import ExitStack

import ml_dtypes
import numpy as np

import concourse.bass as bass
import concourse.tile as tile
from concourse import bacc, mybir
from concourse.alu_op_type import AluOpType
from concourse.bass_utils import run_bass_kernel_spmd

L, H, KVH, D = 2, 32, 8, 64
DM, FF = 2048, 8192
B, S = 1, 1024
EPS, THETA = 1e-5, 10000.0
NCORES = 8
QH = H // NCORES            # 4 q heads per core
QC = QH * D                 # 256 q cols per core
FFS = FF // NCORES          # 1024 ff dims per core
KT = DM // 128              # 16 contraction tiles over DM
FMT = FFS // 128            # 8 f tiles
NEG = -1.0e30

F32 = mybir.dt.float32
BF16 = mybir.dt.bfloat16
AF = mybir.ActivationFunctionType


def _bcast_ap(dram_ap, parts, free, offset_elems=0):
    """DRAM AP replicating a row across `parts` partitions."""
    return bass.AP(
        tensor=dram_ap.tensor,
        offset=dram_ap.offset + offset_elems,
        ap=[[0, parts], [1, free]],
    )


def build(reps=1, debug_stage=None):
    nc = bacc.Bacc(None, target_bir_lowering=False, debug=False, num_devices=NCORES)
    xt_in = nc.dram_tensor("xt", [DM, S], F32, kind="ExternalInput").ap()
    wqkv_in = nc.dram_tensor("wqkv", [L, DM, 384], BF16, kind="ExternalInput").ap()
    wo_in = nc.dram_tensor("wo", [L, QC, DM], BF16, kind="ExternalInput").ap()
    wg_in = nc.dram_tensor("wg", [L, DM, FFS], BF16, kind="ExternalInput").ap()
    wu_in = nc.dram_tensor("wu", [L, DM, FFS], BF16, kind="ExternalInput").ap()
    wd_in = nc.dram_tensor("wd", [L, FFS, DM], BF16, kind="ExternalInput").ap()
    cos_in = nc.dram_tensor("cosr", [128, S], BF16, kind="ExternalInput").ap()
    sin_in = nc.dram_tensor("sinr", [128, S], BF16, kind="ExternalInput").ap()
    mask_in = nc.dram_tensor("mask", [2, 128, 256], F32, kind="ExternalInput").ap()
    y_out = nc.dram_tensor("y", [DM, S], F32, kind="ExternalOutput").ap()

    with tile.TileContext(nc) as tc, ExitStack() as ctx:
        const = ctx.enter_context(tc.tile_pool(name="const", bufs=1))
        xtp = ctx.enter_context(tc.tile_pool(name="xtp", bufs=1))
        xbp = ctx.enter_context(tc.tile_pool(name="xbp", bufs=4))
        wpool = ctx.enter_context(tc.tile_pool(name="wpool", bufs=3))
        wob = ctx.enter_context(tc.tile_pool(name="wob", bufs=1))
        wbig = ctx.enter_context(tc.tile_pool(name="wbig", bufs=2))
        sq = ctx.enter_context(tc.tile_pool(name="sq", bufs=2))
        rp = ctx.enter_context(tc.tile_pool(name="rp", bufs=2))
        attn_sb = ctx.enter_context(tc.tile_pool(name="attn_sb", bufs=1))
        ropet = ctx.enter_context(tc.tile_pool(name="ropet", bufs=1))
        vap = ctx.enter_context(tc.tile_pool(name="vap", bufs=1))
        expp = ctx.enter_context(tc.tile_pool(name="expp", bufs=4))
        smp = ctx.enter_context(tc.tile_pool(name="smp", bufs=2))
        sumsp = ctx.enter_context(tc.tile_pool(name="sumsp", bufs=2))
        stkp = ctx.enter_context(tc.tile_pool(name="stkp", bufs=1))
        evp = ctx.enter_context(tc.tile_pool(name="evp", bufs=3))
        arp = ctx.enter_context(tc.tile_pool(name="arp", bufs=2))
        actp = ctx.enter_context(tc.tile_pool(name="actp", bufs=1))
        mevp = ctx.enter_context(tc.tile_pool(name="mevp", bufs=2))

        dram = ctx.enter_context(tc.tile_pool(name="dram", bufs=2, space="DRAM"))
        ccp = ctx.enter_context(tc.tile_pool(name="ccp", bufs=2, space="DRAM"))

        # ---- persistent constants ----
        onesb = const.tile([128, 1], BF16)
        nc.vector.memset(onesb[:], 1.0)
        cosr = const.tile([128, S], BF16)
        nc.sync.dma_start(cosr[:], cos_in[:])
        sinr = const.tile([128, S], BF16)
        nc.sync.dma_start(sinr[:], sin_in[:])
        mask0 = const.tile([128, 256], F32)
        nc.sync.dma_start(mask0[:], mask_in[0, :, :])
        mask1 = const.tile([128, 256], F32)
        nc.sync.dma_start(mask1[:], mask_in[1, :, :])
        epsb = const.tile([1, 1], F32)
        nc.vector.memset(epsb[:], EPS)

        # residual stream, f32, resident
        xt = [xtp.tile([128, S], F32, tag=f"xt{k}", name=f"xt{k}") for k in range(KT)]

        def load_x():
            for k in range(KT):
                nc.sync.dma_start(xt[k][:], xt_in[k * 128:(k + 1) * 128, :])

        def xb_full(k):
            t = xbp.tile([128, S], BF16, tag="xb")
            nc.vector.tensor_copy(t[:], xt[k][:])
            return t

        def xb_slice(k, sl):
            t = xbp.tile([128, 512], BF16, tag="xbs")
            nc.vector.tensor_copy(t[:], xt[k][:, sl])
            return t

        def rms():
            """r = rsqrt(mean_dm(x^2)+eps) -> (r_bcast [128,S] f32, r_dram [S])"""
            ctx_rms = ExitStack()
            ps_stat = ctx_rms.enter_context(tc.tile_pool(name="ps_stat", bufs=2, space="PSUM"))
            ssum = [ps_stat.tile([1, 512], F32, tag="ssum", name="ssum") for _ in range(2)]
            for k in range(KT):
                xsq = sq.tile([128, S], BF16, tag="xsq")
                nc.scalar.activation(xsq[:], xt[k][:], AF.Square)
                for st in range(2):
                    nc.tensor.matmul(ssum[st][:], onesb[:],
                                     xsq[:, st * 512:(st + 1) * 512],
                                     start=(k == 0), stop=(k == KT - 1))
            rr = rp.tile([1, S], F32, tag="rr", bufs=1)
            for st in range(2):
                rs = rp.tile([1, 512], F32, tag="rs", bufs=1)
                nc.scalar.activation(rs[:], ssum[st][:], AF.Sqrt,
                                     bias=epsb[:], scale=1.0 / DM)
                nc.vector.reciprocal(rr[:, st * 512:(st + 1) * 512], rs[:])
            r_dram = dram.tile([S], F32, tag="r_dram")
            nc.sync.dma_start(r_dram[:], rr[:])
            rb = rp.tile([128, S], F32, tag="rb", bufs=1)
            nc.sync.dma_start(rb[:], _bcast_ap(r_dram, 128, S))
            ctx_rms.close()
            return rb, r_dram

        def attn_block(l):
            rb, r_dram = rms()
            r_cols = rp.tile([128, 8], F32, tag="rcols")
            nc.sync.dma_start(
                r_cols[:],
                bass.AP(tensor=r_dram.tensor, offset=r_dram.offset,
                        ap=[[1, 128], [128, 8]]))

            # ---- q/kT (transposed [cols, s]) and v (natural [s, d]) ----
            ctx_qkv = ExitStack()
            ps_qkv = ctx_qkv.enter_context(tc.tile_pool(name="ps_qkv", bufs=1, space="PSUM"))
            pq = [[ps_qkv.tile([128, 512], F32, tag=f"pq{m}_{st}", name=f"pq{m}_{st}")
                   for st in range(2)] for m in range(3)]
            for k in range(KT):
                wt = wpool.tile([128, 384], BF16, tag="wqkv")
                nc.sync.dma_start(wt[:], wqkv_in[l, k * 128:(k + 1) * 128, :])
                xbk = xb_full(k)
                st_, sp_ = (k == 0), (k == KT - 1)
                for st in range(2):
                    sl = slice(st * 512, (st + 1) * 512)
                    nc.tensor.matmul(pq[0][st][:], wt[:, 0:128], xbk[:, sl], start=st_, stop=sp_)
                    nc.tensor.matmul(pq[1][st][:], wt[:, 128:256], xbk[:, sl], start=st_, stop=sp_)
                    nc.tensor.matmul(pq[2][st][0:64, :], wt[:, 256:320], xbk[:, sl], start=st_, stop=sp_)
            q01 = attn_sb.tile([128, S], BF16, tag="q01")
            q23 = attn_sb.tile([128, S], BF16, tag="q23")
            kt2 = attn_sb.tile([128, S], BF16, tag="kt2")
            for st in range(2):
                sl = slice(st * 512, (st + 1) * 512)
                nc.vector.tensor_tensor(q01[:, sl], pq[0][st][:], rb[:, sl], AluOpType.mult)
                nc.vector.tensor_tensor(q23[:, sl], pq[1][st][:], rb[:, sl], AluOpType.mult)
                nc.vector.tensor_tensor(kt2[0:64, sl], pq[2][st][0:64, :], rb[0:64, sl], AluOpType.mult)
            ctx_qkv.close()
            # ---- v in natural [s, d] layout: one psum tile per 128-row s block
            ctx_v = ExitStack()
            ps_v = ctx_v.enter_context(tc.tile_pool(name="ps_v", bufs=1, space="PSUM"))
            vt = [ps_v.tile([128, 64], F32, tag=f"vt{i}", name=f"vt{i}") for i in range(8)]
            for k in range(KT):
                wvt = wpool.tile([128, 64], BF16, tag="wvt")
                nc.sync.dma_start(wvt[:], wqkv_in[l, k * 128:(k + 1) * 128, 320:384])
                xbk = xb_full(k)
                for sj in range(8):
                    nc.tensor.matmul(vt[sj][:], xbk[:, sj * 128:(sj + 1) * 128], wvt[:],
                                     start=(k == 0), stop=(k == KT - 1))
            va = []
            for sj in range(8):
                v = vap.tile([128, 66], BF16, tag=f"va{sj}", name=f"va{sj}")
                nc.vector.tensor_scalar_mul(v[:, 0:64], vt[sj][:], r_cols[:, sj:sj + 1])
                nc.vector.memset(v[:, 64:65], 1.0)
                va.append(v)
            ctx_v.close()

            # ---- RoPE ----
            def rope(t, nrows):
                rot = ropet.tile([128, S], BF16, tag="rot")
                for h0 in range(0, nrows, 64):
                    nc.sync.dma_start(rot[h0:h0 + 32, :], t[h0 + 32:h0 + 64, :])
                    nc.sync.dma_start(rot[h0 + 32:h0 + 64, :], t[h0:h0 + 32, :])
                t1 = ropet.tile([128, S], BF16, tag="t1")
                t2 = ropet.tile([128, S], BF16, tag="t2")
                nc.vector.tensor_tensor(t1[0:nrows, :], t[0:nrows, :], cosr[0:nrows, :], AluOpType.mult)
                nc.vector.tensor_tensor(t2[0:nrows, :], rot[0:nrows, :], sinr[0:nrows, :], AluOpType.mult)
                nc.vector.tensor_add(t[0:nrows, :], t1[0:nrows, :], t2[0:nrows, :])
            rope(q01, 128)
            rope(q23, 128)
            rope(kt2, 64)
            nc.sync.dma_start(kt2[64:128, :], kt2[0:64, :])  # duplicate kv head

            # ---- attention (4 heads; causal over 128-wide j blocks) ----
            stk0 = stkp.tile([128, S], BF16, tag="stk0")
            stk1 = stkp.tile([128, S], BF16, tag="stk1")
            sinv_dram = dram.tile([4, S], F32, tag="sinv")
            ctx_att = ExitStack()
            ps_sc = ctx_att.enter_context(tc.tile_pool(name="ps_sc", bufs=3, space="PSUM"))
            ps_at = ctx_att.enter_context(tc.tile_pool(name="ps_at", bufs=2, space="PSUM"))
            for h in range(4):
                qt = (q01, q23)[h // 2]
                rows = slice(64 * (h % 2), 64 * (h % 2) + 64)
                stk = (stk0, stk1)[h // 2]
                odd = h % 2 == 1
                for it in range(4):
                    isl = slice(it * 256, (it + 1) * 256)
                    aps = ps_at.tile([66, 256], F32, tag="aps")
                    for j in range(2 * it + 2):
                        sps = ps_sc.tile([128, 256], F32, tag="sps")
                        nc.tensor.matmul(sps[:], kt2[rows, j * 128:(j + 1) * 128],
                                         qt[rows, isl], start=True, stop=True)
                        e = expp.tile([128, 256], BF16, tag="e")
                        if j >= 2 * it:
                            sm = smp.tile([128, 256], F32, tag="sm")
                            nc.vector.tensor_add(sm[:], sps[:], (mask0, mask1)[j - 2 * it][:])
                            nc.scalar.activation(e[:], sm[:], AF.Exp)
                        else:
                            nc.scalar.activation(e[:], sps[:], AF.Exp)
                        nc.tensor.matmul(aps[0:65, :], va[j][:, 0:65], e[:],
                                         start=(j == 0), stop=(j == 2 * it + 1))
                    # row 64 of aps = sum_j exp ; rows 0:64 = unnormalized attnT
                    sv = sumsp.tile([66, 256], F32, tag="sv")
                    nc.vector.reciprocal(sv[64:65, :], aps[64:65, :])
                    nc.sync.dma_start(sinv_dram[h, isl], sv[64:65, :])
                    if odd:
                        tmp = evp.tile([66, 256], BF16, tag="oddh")
                        nc.vector.tensor_copy(tmp[0:64, :], aps[0:64, :])
                        nc.sync.dma_start(stk[64:128, isl], tmp[0:64, :])
                    else:
                        nc.vector.tensor_copy(stk[0:64, isl], aps[0:64, :])
            ctx_att.close()
            # normalize: stk *= 1/sums (broadcast per 64-row head block)
            for t, h0, h1 in ((stk0, 0, 1), (stk1, 2, 3)):
                sb = rp.tile([128, S], F32, tag="sinvb")
                nc.sync.dma_start(sb[0:64, :], _bcast_ap(sinv_dram, 64, S, h0 * S))
                nc.sync.dma_start(sb[64:128, :], _bcast_ap(sinv_dram, 64, S, h1 * S))
                nc.vector.tensor_tensor(t[:], t[:], sb[:], AluOpType.mult)

            if debug_stage == f"stk{l}":
                for ti, t in enumerate((stk0, stk1)):
                    pr = evp.tile([128, S], F32, tag="dbg")
                    nc.vector.tensor_copy(pr[:], t[:])
                    nc.sync.dma_start(y_out[ti * 128:(ti + 1) * 128, :], pr[:])
                for sj in range(8):
                    pr2 = evp.tile([128, 66], F32, tag="dbg2")
                    nc.vector.tensor_copy(pr2[:], va[sj][:])
                    nc.sync.dma_start(y_out[256 + sj * 128:256 + (sj + 1) * 128, 0:66], pr2[:])
                pr3 = evp.tile([128, S], F32, tag="dbg")
                nc.vector.tensor_copy(pr3[:], q01[:])
                nc.sync.dma_start(y_out[1536:1664, :], pr3[:])
                pr4 = evp.tile([128, S], F32, tag="dbg")
                nc.vector.tensor_copy(pr4[:], kt2[:])
                nc.sync.dma_start(y_out[1664:1792, :], pr4[:])
                pr5 = evp.tile([128, 8], F32, tag="dbg3")
                nc.vector.tensor_copy(pr5[:], r_cols[:])
                nc.sync.dma_start(y_out[1792:1920, 0:8], pr5[:])
                return
            # ---- wo projection -> partial [DM, S] -> AllReduce -> residual ----
            wo0 = wob.tile([128, DM], BF16, tag="wo0")
            nc.sync.dma_start(wo0[:], wo_in[l, 0:128, :])
            wo1 = wob.tile([128, DM], BF16, tag="wo1")
            nc.sync.dma_start(wo1[:], wo_in[l, 128:256, :])
            cc_in = ccp.tile([DM, S], F32, tag="cc_in")
            cc_out = ccp.tile([DM, S], F32, tag="cc_out")
            ctx_wo = ExitStack()
            ps_wo = ctx_wo.enter_context(tc.tile_pool(name="ps_wo", bufs=4, space="PSUM"))
            for dmm in range(KT):
                dsl = slice(dmm * 128, (dmm + 1) * 128)
                for st in range(2):
                    sl = slice(st * 512, (st + 1) * 512)
                    wops = ps_wo.tile([128, 512], F32, tag="wops")
                    nc.tensor.matmul(wops[:], wo0[:, dsl], stk0[:, sl], start=True, stop=False)
                    nc.tensor.matmul(wops[:], wo1[:, dsl], stk1[:, sl], start=False, stop=True)
                    pr = evp.tile([128, 512], F32, tag="pr")
                    nc.vector.tensor_copy(pr[:], wops[:])
                    nc.sync.dma_start(cc_in[dsl, sl], pr[:])
            ctx_wo.close()
            nc.gpsimd.collective_compute(
                "AllReduce", AluOpType.add,
                replica_groups=[list(range(NCORES))],
                ins=[cc_in[:].opt()], outs=[cc_out[:].opt()])
            for k in range(KT):
                ar = arp.tile([128, S], F32, tag="ar")
                nc.sync.dma_start(ar[:], cc_out[k * 128:(k + 1) * 128, :])
                nc.vector.tensor_add(xt[k][:], xt[k][:], ar[:])

        def mlp_block(l, last):
            rb, _ = rms()
            if not last:
                cc_in = ccp.tile([DM, S], F32, tag="cc_in")
                cc_out = ccp.tile([DM, S], F32, tag="cc_out")
            for st in range(2):
                sl = slice(st * 512, (st + 1) * 512)
                ctx_mlp = ExitStack()
                ps_mlp = ctx_mlp.enter_context(tc.tile_pool(name="ps_mlp", bufs=1, space="PSUM"))
                acts = []
                for name, w_in in (("g", wg_in), ("u", wu_in)):
                    ps = [ps_mlp.tile([128, 512], F32, tag=f"mlp{fm}", name=f"mlp{fm}") for fm in range(FMT)]
                    for k in range(KT):
                        wt = wbig.tile([128, FFS], BF16, tag="wgu")
                        nc.sync.dma_start(wt[:], w_in[l, k * 128:(k + 1) * 128, :])
                        xbs = xb_slice(k, sl)
                        for fm in range(FMT):
                            nc.tensor.matmul(ps[fm][:], wt[:, fm * 128:(fm + 1) * 128],
                                             xbs[:], start=(k == 0), stop=(k == KT - 1))
                    row = []
                    for fm in range(FMT):
                        t = mevp.tile([128, 512], BF16, tag=f"ev{name}")
                        nc.vector.tensor_tensor(t[:], ps[fm][:], rb[:, sl], AluOpType.mult)
                        if name == "g":
                            t2 = actp.tile([128, 512], BF16, tag=f"sil{fm}")
                            nc.scalar.activation(t2[:], t[:], AF.Silu)
                            t = t2
                        row.append(t)
                    acts.append(row)
                prod = []
                for fm in range(FMT):
                    t = actp.tile([128, 512], BF16, tag=f"act{fm}")
                    nc.vector.tensor_tensor(t[:], acts[0][fm][:], acts[1][fm][:], AluOpType.mult)
                    prod.append(t)
                for grp in range(2):
                    dps = [ps_mlp.tile([128, 512], F32, tag=f"mlp{d}", name=f"mlpd{d}") for d in range(FMT)]
                    for fk in range(FMT):
                        wdt = wbig.tile([128, 1024], BF16, tag="wdt")
                        nc.sync.dma_start(
                            wdt[:], wd_in[l, fk * 128:(fk + 1) * 128,
                                          grp * 1024:(grp + 1) * 1024])
                        for dmm in range(8):
                            nc.tensor.matmul(dps[dmm][:], wdt[:, dmm * 128:(dmm + 1) * 128],
                                             prod[fk][:], start=(fk == 0), stop=(fk == FMT - 1))
                    for dmm in range(8):
                        kk = grp * 8 + dmm
                        dsl = slice(kk * 128, (kk + 1) * 128)
                        pr = evp.tile([128, 512], F32, tag="pr")
                        if last:
                            nc.vector.scalar_tensor_tensor(
                                pr[:], xt[kk][:, sl], 1.0 / NCORES, dps[dmm][:],
                                AluOpType.mult, AluOpType.add)
                            nc.sync.dma_start(y_out[dsl, sl], pr[:])
                        else:
                            nc.vector.tensor_copy(pr[:], dps[dmm][:])
                            nc.sync.dma_start(cc_in[dsl, sl], pr[:])
                ctx_mlp.close()
            if not last:
                nc.gpsimd.collective_compute(
                    "AllReduce", AluOpType.add,
                    replica_groups=[list(range(NCORES))],
                    ins=[cc_in[:].opt()], outs=[cc_out[:].opt()])
                for k in range(KT):
                    ar = arp.tile([128, S], F32, tag="ar")
                    nc.sync.dma_start(ar[:], cc_out[k * 128:(k + 1) * 128, :])
                    nc.vector.tensor_add(xt[k][:], xt[k][:], ar[:])

        def dump_x():
            for k in range(KT):
                pr = evp.tile([128, S], F32, tag="dbg")
                nc.vector.tensor_copy(pr[:], xt[k][:])
                nc.sync.dma_start(y_out[k * 128:(k + 1) * 128, :], pr[:])

        stages = []
        for l in range(L):
            stages.append((f"attn{l}", attn_block, l))
            stages.append((f"mlp{l}", mlp_block, l))
        for _ in range(reps):
            load_x()
            for sname, fn, l in stages:
                if fn is mlp_block:
                    fn(l, last=(l == L - 1 and debug_stage is None))
                else:
                    fn(l)
                    if debug_stage == f"stk{l}":
                        break
                if debug_stage == sname:
                    dump_x()
                    break

    nc.compile()
    return nc


def make_inputs(input_ids, embed, wq, wk, wv, wo, wgate, wup, wdown, ln1, ln2):
    """host-side prep: embedding gather, shard + fold norm gains/scale into weights."""
    f32 = np.float32
    bf = ml_dtypes.bfloat16
    x = np.asarray(embed, f32)[np.asarray(input_ids)[0]]      # (S, DM)
    xt = np.ascontiguousarray(x.T)                            # (DM, S)

    inv_freq = 1.0 / (THETA ** (np.arange(0, D, 2, dtype=f32) / D))
    freqs = np.arange(S, dtype=f32)[:, None] * inv_freq[None, :]    # (S, 32)
    emb = np.concatenate([freqs, freqs], axis=1)                    # (S, D)
    cosT = np.cos(emb).T.astype(f32)                                # (D, S)
    sinT = np.sin(emb).T.astype(f32)
    sinT_signed = sinT.copy()
    sinT_signed[: D // 2] *= -1.0
    cos_rep = np.concatenate([cosT, cosT], axis=0).astype(bf)       # (128, S)
    sin_rep = np.concatenate([sinT_signed, sinT_signed], axis=0).astype(bf)

    mask = np.zeros((2, 128, 256), f32)
    jj = np.arange(128)[:, None]
    ii = np.arange(256)[None, :]
    for o in range(2):
        mask[o] = np.where(128 * o + jj <= ii, 0.0, NEG)

    scale_q = 1.0 / math.sqrt(D)
    in_maps = []
    for c in range(NCORES):
        wqkv = np.empty((L, DM, 384), f32)
        wo_c = np.empty((L, QC, DM), f32)
        wg_c = np.empty((L, DM, FFS), f32)
        wu_c = np.empty((L, DM, FFS), f32)
        wd_c = np.empty((L, FFS, DM), f32)
        for l in range(L):
            g1 = np.asarray(ln1[l], f32)[:, None]
            g2 = np.asarray(ln2[l], f32)[:, None]
            wqkv[l, :, :QC] = np.asarray(wq[l], f32)[:, c * QC:(c + 1) * QC] * g1 * scale_q
            wqkv[l, :, QC:QC + D] = np.asarray(wk[l], f32)[:, c * D:(c + 1) * D] * g1
            wqkv[l, :, QC + D:] = np.asarray(wv[l], f32)[:, c * D:(c + 1) * D] * g1
            wo_c[l] = np.asarray(wo[l], f32)[c * QC:(c + 1) * QC, :]
            wg_c[l] = np.asarray(wgate[l], f32)[:, c * FFS:(c + 1) * FFS] * g2
            wu_c[l] = np.asarray(wup[l], f32)[:, c * FFS:(c + 1) * FFS] * g2
            wd_c[l] = np.asarray(wdown[l], f32)[c * FFS:(c + 1) * FFS, :]
        in_maps.append({
            "xt": xt, "wqkv": wqkv.astype(bf),
            "wo": wo_c.astype(bf), "wg": wg_c.astype(bf), "wu": wu_c.astype(bf),
            "wd": wd_c.astype(bf), "cosr": cos_rep, "sinr": sin_rep, "mask": mask,
        })
    return in_maps


_NC_CACHE = {}


def kernel(**inputs) -> np.ndarray:
    if 1 not in _NC_CACHE:
        _NC_CACHE[1] = build(reps=1)
    nc = _NC_CACHE[1]
    in_maps = make_inputs(**inputs)
    res = run_bass_kernel_spmd(nc, in_maps, list(range(NCORES)))
    y = np.zeros((DM, S), np.float64)
    for c in range(NCORES):
        y += res.results[c]["y"].astype(np.float64)
    return np.ascontiguousarray(y.T.astype(np.float32)).reshape(B, S, DM)


# revision 10
# speedup vs baseline: 391.3441x; 391.3441x over previous
"""Llama trunk (2 layers, before final norm) on 8 trn2 cores.

Sharding: Megatron tensor-parallel over 8 cores.
  - attention: 4 q-heads + 1 kv-head per core (GQA group stays local)
  - MLP: 1024 of 8192 intermediate dims per core
  - residual stream x replicated in f32; matmul operands bf16
Layout: transposed on device: xT is [DM(part), S(free)] so every matmul
contracts over the partition axis.  Final layer's down-proj partials are
reduced on the host (saves one AllReduce); the other 3 AllReduces run
on-device through DRAM bounce buffers.  RMSNorm gains and 1/sqrt(D) are
folded into the weights on the host.
"""
import math
from contextlib import ExitStack

import ml_dtypes
import numpy as np

import concourse.bass as bass
import concourse.tile as tile
from concourse import bacc, mybir
from concourse.alu_op_type import AluOpType
from concourse.bass_utils import run_bass_kernel_spmd

L, H, KVH, D = 2, 32, 8, 64
DM, FF = 2048, 8192
B, S = 1, 1024
EPS, THETA = 1e-5, 10000.0
NCORES = 8
QH = H // NCORES            # 4 q heads per core
QC = QH * D                 # 256 q cols per core
FFS = FF // NCORES          # 1024 ff dims per core
KT = DM // 128              # 16 contraction tiles over DM
FMT = FFS // 128            # 8 f tiles
NEG = -1.0e30

F32 = mybir.dt.float32
F32R = mybir.dt.float32r
BF16 = mybir.dt.bfloat16
AF = mybir.ActivationFunctionType


def _bcast_ap(dram_ap, parts, free, offset_elems=0):
    """DRAM AP replicating a row across `parts` partitions."""
    return bass.AP(
        tensor=dram_ap.tensor,
        offset=dram_ap.offset + offset_elems,
        ap=[[0, parts], [1, free]],
    )


def build(reps=1, debug_stage=None):
    nc = bacc.Bacc(None, target_bir_lowering=False, debug=False, num_devices=NCORES)
    xt_in = nc.dram_tensor("xt", [DM, S], F32, kind="ExternalInput").ap()
    wqkv_in = nc.dram_tensor("wqkv", [L, DM, 384], BF16, kind="ExternalInput").ap()
    wo_in = nc.dram_tensor("wo", [L, QC, DM], BF16, kind="ExternalInput").ap()
    wg_in = nc.dram_tensor("wg", [L, DM, FFS], BF16, kind="ExternalInput").ap()
    wu_in = nc.dram_tensor("wu", [L, DM, FFS], BF16, kind="ExternalInput").ap()
    wd_in = nc.dram_tensor("wd", [L, FFS, DM], BF16, kind="ExternalInput").ap()
    cos_in = nc.dram_tensor("cosr", [128, S], F32R, kind="ExternalInput").ap()
    sin_in = nc.dram_tensor("sinr", [128, S], F32R, kind="ExternalInput").ap()
    mask_in = nc.dram_tensor("mask", [2, 128, 256], F32, kind="ExternalInput").ap()
    y_out = nc.dram_tensor("y", [DM, S], F32, kind="ExternalOutput").ap()

    with tile.TileContext(nc) as tc, ExitStack() as ctx:
        const = ctx.enter_context(tc.tile_pool(name="const", bufs=1))
        xtp = ctx.enter_context(tc.tile_pool(name="xtp", bufs=1))
        xbp = ctx.enter_context(tc.tile_pool(name="xbp", bufs=4))
        wpool = ctx.enter_context(tc.tile_pool(name="wpool", bufs=3))
        wob = ctx.enter_context(tc.tile_pool(name="wob", bufs=1))
        wbig = ctx.enter_context(tc.tile_pool(name="wbig", bufs=2))
        sq = ctx.enter_context(tc.tile_pool(name="sq", bufs=2))
        rp = ctx.enter_context(tc.tile_pool(name="rp", bufs=2))
        attn_sb = ctx.enter_context(tc.tile_pool(name="attn_sb", bufs=1))
        ropet = ctx.enter_context(tc.tile_pool(name="ropet", bufs=1))
        vap = ctx.enter_context(tc.tile_pool(name="vap", bufs=1))
        expp = ctx.enter_context(tc.tile_pool(name="expp", bufs=4))
        smp = ctx.enter_context(tc.tile_pool(name="smp", bufs=2))
        sumsp = ctx.enter_context(tc.tile_pool(name="sumsp", bufs=2))
        stkp = ctx.enter_context(tc.tile_pool(name="stkp", bufs=1))
        evp = ctx.enter_context(tc.tile_pool(name="evp", bufs=3))
        arp = ctx.enter_context(tc.tile_pool(name="arp", bufs=2))
        actp = ctx.enter_context(tc.tile_pool(name="actp", bufs=1))
        mevp = ctx.enter_context(tc.tile_pool(name="mevp", bufs=2))

        dram = ctx.enter_context(tc.tile_pool(name="dram", bufs=2, space="DRAM"))
        ccp = ctx.enter_context(tc.tile_pool(name="ccp", bufs=2, space="DRAM"))

        # ---- persistent constants ----
        onesb = const.tile([128, 1], BF16)
        nc.vector.memset(onesb[:], 1.0)
        cosr = const.tile([128, S], F32R)
        nc.sync.dma_start(cosr[:], cos_in[:])
        sinr = const.tile([128, S], F32R)
        nc.sync.dma_start(sinr[:], sin_in[:])
        mask0 = const.tile([128, 256], F32)
        nc.sync.dma_start(mask0[:], mask_in[0, :, :])
        mask1 = const.tile([128, 256], F32)
        nc.sync.dma_start(mask1[:], mask_in[1, :, :])
        epsb = const.tile([1, 1], F32)
        nc.vector.memset(epsb[:], EPS)

        # residual stream, f32, resident
        xt = [xtp.tile([128, S], F32, tag=f"xt{k}", name=f"xt{k}") for k in range(KT)]

        def load_x():
            for k in range(KT):
                nc.sync.dma_start(xt[k][:], xt_in[k * 128:(k + 1) * 128, :])

        def xb_full(k):
            t = xbp.tile([128, S], BF16, tag="xb")
            nc.vector.tensor_copy(t[:], xt[k][:])
            return t

        def xb_slice(k, sl):
            t = xbp.tile([128, 512], BF16, tag="xbs")
            nc.vector.tensor_copy(t[:], xt[k][:, sl])
            return t

        def rms():
            """r = rsqrt(mean_dm(x^2)+eps) -> (r_bcast [128,S] f32, r_dram [S])"""
            ctx_rms = ExitStack()
            ps_stat = ctx_rms.enter_context(tc.tile_pool(name="ps_stat", bufs=2, space="PSUM"))
            ssum = [ps_stat.tile([1, 512], F32, tag="ssum", name="ssum") for _ in range(2)]
            for k in range(KT):
                xsq = sq.tile([128, S], BF16, tag="xsq")
                nc.scalar.activation(xsq[:], xt[k][:], AF.Square)
                for st in range(2):
                    nc.tensor.matmul(ssum[st][:], onesb[:],
                                     xsq[:, st * 512:(st + 1) * 512],
                                     start=(k == 0), stop=(k == KT - 1))
            rr = rp.tile([1, S], F32, tag="rr", bufs=1)
            for st in range(2):
                rs = rp.tile([1, 512], F32, tag="rs", bufs=1)
                nc.scalar.activation(rs[:], ssum[st][:], AF.Sqrt,
                                     bias=epsb[:], scale=1.0 / DM)
                nc.vector.reciprocal(rr[:, st * 512:(st + 1) * 512], rs[:])
            r_dram = dram.tile([S], F32, tag="r_dram")
            nc.sync.dma_start(r_dram[:], rr[:])
            rb = rp.tile([128, S], F32, tag="rb", bufs=1)
            nc.sync.dma_start(rb[:], _bcast_ap(r_dram, 128, S))
            ctx_rms.close()
            return rb, r_dram

        def attn_block(l):
            rb, r_dram = rms()
            r_cols = rp.tile([128, 8], F32, tag="rcols")
            nc.sync.dma_start(
                r_cols[:],
                bass.AP(tensor=r_dram.tensor, offset=r_dram.offset,
                        ap=[[1, 128], [128, 8]]))

            # ---- q/kT (transposed [cols, s]) and v (natural [s, d]) ----
            ctx_qkv = ExitStack()
            ps_qkv = ctx_qkv.enter_context(tc.tile_pool(name="ps_qkv", bufs=1, space="PSUM"))
            pq = [[ps_qkv.tile([128, 512], F32, tag=f"pq{m}_{st}", name=f"pq{m}_{st}")
                   for st in range(2)] for m in range(3)]
            for k in range(KT):
                wt = wpool.tile([128, 384], BF16, tag="wqkv")
                nc.sync.dma_start(wt[:], wqkv_in[l, k * 128:(k + 1) * 128, :])
                xbk = xb_full(k)
                st_, sp_ = (k == 0), (k == KT - 1)
                for st in range(2):
                    sl = slice(st * 512, (st + 1) * 512)
                    nc.tensor.matmul(pq[0][st][:], wt[:, 0:128], xbk[:, sl], start=st_, stop=sp_)
                    nc.tensor.matmul(pq[1][st][:], wt[:, 128:256], xbk[:, sl], start=st_, stop=sp_)
                    nc.tensor.matmul(pq[2][st][0:64, :], wt[:, 256:320], xbk[:, sl], start=st_, stop=sp_)
            q01 = attn_sb.tile([128, S], F32R, tag="q01")
            q23 = attn_sb.tile([128, S], F32R, tag="q23")
            kt2 = attn_sb.tile([128, S], F32R, tag="kt2")
            for st in range(2):
                sl = slice(st * 512, (st + 1) * 512)
                nc.vector.tensor_tensor(q01[:, sl], pq[0][st][:], rb[:, sl], AluOpType.mult)
                nc.vector.tensor_tensor(q23[:, sl], pq[1][st][:], rb[:, sl], AluOpType.mult)
                nc.vector.tensor_tensor(kt2[0:64, sl], pq[2][st][0:64, :], rb[0:64, sl], AluOpType.mult)
            ctx_qkv.close()
            # ---- v in natural [s, d] layout: one psum tile per 128-row s block
            ctx_v = ExitStack()
            ps_v = ctx_v.enter_context(tc.tile_pool(name="ps_v", bufs=1, space="PSUM"))
            vt = [ps_v.tile([128, 64], F32, tag=f"vt{i}", name=f"vt{i}") for i in range(8)]
            for k in range(KT):
                wvt = wpool.tile([128, 64], BF16, tag="wvt")
                nc.sync.dma_start(wvt[:], wqkv_in[l, k * 128:(k + 1) * 128, 320:384])
                xbk = xb_full(k)
                for sj in range(8):
                    nc.tensor.matmul(vt[sj][:], xbk[:, sj * 128:(sj + 1) * 128], wvt[:],
                                     start=(k == 0), stop=(k == KT - 1))
            va = []
            for sj in range(8):
                v = vap.tile([128, 66], F32R, tag=f"va{sj}", name=f"va{sj}")
                nc.vector.tensor_scalar_mul(v[:, 0:64], vt[sj][:], r_cols[:, sj:sj + 1])
                nc.vector.tensor_copy(v[:, 64:65], onesb[:])
                va.append(v)
            ctx_v.close()

            # ---- RoPE ----
            def rope(t, nrows):
                rot = ropet.tile([128, S], F32R, tag="rot")
                for h0 in range(0, nrows, 64):
                    nc.sync.dma_start(rot[h0:h0 + 32, :], t[h0 + 32:h0 + 64, :])
                    nc.sync.dma_start(rot[h0 + 32:h0 + 64, :], t[h0:h0 + 32, :])
                t1 = ropet.tile([128, S], F32R, tag="t1")
                t2 = ropet.tile([128, S], F32R, tag="t2")
                nc.vector.tensor_tensor(t1[0:nrows, :], t[0:nrows, :], cosr[0:nrows, :], AluOpType.mult)
                nc.vector.tensor_tensor(t2[0:nrows, :], rot[0:nrows, :], sinr[0:nrows, :], AluOpType.mult)
                nc.vector.tensor_add(t[0:nrows, :], t1[0:nrows, :], t2[0:nrows, :])
            rope(q01, 128)
            rope(q23, 128)
            rope(kt2, 64)
            nc.sync.dma_start(kt2[64:128, :], kt2[0:64, :])  # duplicate kv head

            # ---- attention (4 heads; causal over 128-wide j blocks) ----
            stk0 = stkp.tile([128, S], BF16, tag="stk0")
            stk1 = stkp.tile([128, S], BF16, tag="stk1")
            sinv_dram = dram.tile([4, S], F32, tag="sinv")
            ctx_att = ExitStack()
            ps_sc = ctx_att.enter_context(tc.tile_pool(name="ps_sc", bufs=3, space="PSUM"))
            ps_at = ctx_att.enter_context(tc.tile_pool(name="ps_at", bufs=2, space="PSUM"))
            for h in range(4):
                qt = (q01, q23)[h // 2]
                rows = slice(64 * (h % 2), 64 * (h % 2) + 64)
                stk = (stk0, stk1)[h // 2]
                odd = h % 2 == 1
                for it in range(4):
                    isl = slice(it * 256, (it + 1) * 256)
                    aps = ps_at.tile([66, 256], F32, tag="aps")
                    for j in range(2 * it + 2):
                        sps = ps_sc.tile([128, 256], F32, tag="sps")
                        nc.tensor.matmul(sps[:], kt2[rows, j * 128:(j + 1) * 128],
                                         qt[rows, isl], start=True, stop=True)
                        e = expp.tile([128, 256], F32R, tag="e")
                        if j >= 2 * it:
                            sm = smp.tile([128, 256], F32, tag="sm")
                            nc.vector.tensor_add(sm[:], sps[:], (mask0, mask1)[j - 2 * it][:])
                            nc.scalar.activation(e[:], sm[:], AF.Exp)
                        else:
                            nc.scalar.activation(e[:], sps[:], AF.Exp)
                        nc.tensor.matmul(aps[0:65, :], va[j][:, 0:65], e[:],
                                         start=(j == 0), stop=(j == 2 * it + 1))
                    # row 64 of aps = sum_j exp ; rows 0:64 = unnormalized attnT
                    sv = sumsp.tile([66, 256], F32, tag="sv")
                    nc.vector.reciprocal(sv[64:65, :], aps[64:65, :])
                    nc.sync.dma_start(sinv_dram[h, isl], sv[64:65, :])
                    if odd:
                        tmp = evp.tile([66, 256], BF16, tag="oddh")
                        nc.vector.tensor_copy(tmp[0:64, :], aps[0:64, :])
                        nc.sync.dma_start(stk[64:128, isl], tmp[0:64, :])
                    else:
                        nc.vector.tensor_copy(stk[0:64, isl], aps[0:64, :])
            ctx_att.close()
            # normalize: stk *= 1/sums (broadcast per 64-row head block)
            for t, h0, h1 in ((stk0, 0, 1), (stk1, 2, 3)):
                sb = rp.tile([128, S], F32, tag="sinvb")
                nc.sync.dma_start(sb[0:64, :], _bcast_ap(sinv_dram, 64, S, h0 * S))
                nc.sync.dma_start(sb[64:128, :], _bcast_ap(sinv_dram, 64, S, h1 * S))
                nc.vector.tensor_tensor(t[:], t[:], sb[:], AluOpType.mult)

            if debug_stage == f"stk{l}":
                for ti, t in enumerate((stk0, stk1)):
                    pr = evp.tile([128, S], F32, tag="dbg")
                    nc.vector.tensor_copy(pr[:], t[:])
                    nc.sync.dma_start(y_out[ti * 128:(ti + 1) * 128, :], pr[:])
                for sj in range(8):
                    pr2 = evp.tile([128, 66], F32, tag="dbg2")
                    nc.vector.tensor_copy(pr2[:], va[sj][:])
                    nc.sync.dma_start(y_out[256 + sj * 128:256 + (sj + 1) * 128, 0:66], pr2[:])
                pr3 = evp.tile([128, S], F32, tag="dbg")
                nc.vector.tensor_copy(pr3[:], q01[:])
                nc.sync.dma_start(y_out[1536:1664, :], pr3[:])
                pr4 = evp.tile([128, S], F32, tag="dbg")
                nc.vector.tensor_copy(pr4[:], kt2[:])
                nc.sync.dma_start(y_out[1664:1792, :], pr4[:])
                pr5 = evp.tile([128, 8], F32, tag="dbg3")
                nc.vector.tensor_copy(pr5[:], r_cols[:])
                nc.sync.dma_start(y_out[1792:1920, 0:8], pr5[:])
                return
            # ---- wo projection -> partial [DM, S] -> AllReduce -> residual ----
            wo0 = wob.tile([128, DM], BF16, tag="wo0")
            nc.sync.dma_start(wo0[:], wo_in[l, 0:128, :])
            wo1 = wob.tile([128, DM], BF16, tag="wo1")
            nc.sync.dma_start(wo1[:], wo_in[l, 128:256, :])
            cc_in = ccp.tile([DM, S], F32, tag="cc_in")
            cc_out = ccp.tile([DM, S], F32, tag="cc_out", addr_space="Shared")
            ctx_wo = ExitStack()
            ps_wo = ctx_wo.enter_context(tc.tile_pool(name="ps_wo", bufs=4, space="PSUM"))
            for dmm in range(KT):
                dsl = slice(dmm * 128, (dmm + 1) * 128)
                for st in range(2):
                    sl = slice(st * 512, (st + 1) * 512)
                    wops = ps_wo.tile([128, 512], F32, tag="wops")
                    nc.tensor.matmul(wops[:], wo0[:, dsl], stk0[:, sl], start=True, stop=False)
                    nc.tensor.matmul(wops[:], wo1[:, dsl], stk1[:, sl], start=False, stop=True)
                    pr = evp.tile([128, 512], F32, tag="pr")
                    nc.vector.tensor_copy(pr[:], wops[:])
                    nc.sync.dma_start(cc_in[dsl, sl], pr[:])
            ctx_wo.close()
            nc.gpsimd.collective_compute(
                "AllReduce", AluOpType.add,
                replica_groups=[list(range(NCORES))],
                ins=[cc_in[:].opt()], outs=[cc_out[:].opt()])
            for k in range(KT):
                ar = arp.tile([128, S], F32, tag="ar")
                nc.sync.dma_start(ar[:], cc_out[k * 128:(k + 1) * 128, :])
                nc.vector.tensor_add(xt[k][:], xt[k][:], ar[:])

        def mlp_block(l, last):
            rb, _ = rms()
            if not last:
                cc_in = ccp.tile([DM, S], F32, tag="cc_in")
                cc_out = ccp.tile([DM, S], F32, tag="cc_out", addr_space="Shared")
            for st in range(2):
                sl = slice(st * 512, (st + 1) * 512)
                ctx_mlp = ExitStack()
                ps_mlp = ctx_mlp.enter_context(tc.tile_pool(name="ps_mlp", bufs=1, space="PSUM"))
                acts = []
                for name, w_in in (("g", wg_in), ("u", wu_in)):
                    ps = [ps_mlp.tile([128, 512], F32, tag=f"mlp{fm}", name=f"mlp{fm}") for fm in range(FMT)]
                    for k in range(KT):
                        wt = wbig.tile([128, FFS], BF16, tag="wgu")
                        nc.sync.dma_start(wt[:], w_in[l, k * 128:(k + 1) * 128, :])
                        xbs = xb_slice(k, sl)
                        for fm in range(FMT):
                            nc.tensor.matmul(ps[fm][:], wt[:, fm * 128:(fm + 1) * 128],
                                             xbs[:], start=(k == 0), stop=(k == KT - 1))
                    row = []
                    for fm in range(FMT):
                        t = mevp.tile([128, 512], BF16, tag=f"ev{name}")
                        nc.vector.tensor_tensor(t[:], ps[fm][:], rb[:, sl], AluOpType.mult)
                        if name == "g":
                            t2 = actp.tile([128, 512], BF16, tag=f"sil{fm}")
                            nc.scalar.activation(t2[:], t[:], AF.Silu)
                            t = t2
                        row.append(t)
                    acts.append(row)
                prod = []
                for fm in range(FMT):
                    t = actp.tile([128, 512], BF16, tag=f"act{fm}")
                    nc.vector.tensor_tensor(t[:], acts[0][fm][:], acts[1][fm][:], AluOpType.mult)
                    prod.append(t)
                for grp in range(2):
                    dps = [ps_mlp.tile([128, 512], F32, tag=f"mlp{d}", name=f"mlpd{d}") for d in range(FMT)]
                    for fk in range(FMT):
                        wdt = wbig.tile([128, 1024], BF16, tag="wdt")
                        nc.sync.dma_start(
                            wdt[:], wd_in[l, fk * 128:(fk + 1) * 128,
                                          grp * 1024:(grp + 1) * 1024])
                        for dmm in range(8):
                            nc.tensor.matmul(dps[dmm][:], wdt[:, dmm * 128:(dmm + 1) * 128],
                                             prod[fk][:], start=(fk == 0), stop=(fk == FMT - 1))
                    for dmm in range(8):
                        kk = grp * 8 + dmm
                        dsl = slice(kk * 128, (kk + 1) * 128)
                        pr = evp.tile([128, 512], F32, tag="pr")
                        if last:
                            nc.vector.scalar_tensor_tensor(
                                pr[:], xt[kk][:, sl], 1.0 / NCORES, dps[dmm][:],
                                AluOpType.mult, AluOpType.add)
                            nc.sync.dma_start(y_out[dsl, sl], pr[:])
                        else:
                            nc.vector.tensor_copy(pr[:], dps[dmm][:])
                            nc.sync.dma_start(cc_in[dsl, sl], pr[:])
                ctx_mlp.close()
            if not last:
                nc.gpsimd.collective_compute(
                    "AllReduce", AluOpType.add,
                    replica_groups=[list(range(NCORES))],
                    ins=[cc_in[:].opt()], outs=[cc_out[:].opt()])
                for k in range(KT):
                    ar = arp.tile([128, S], F32, tag="ar")
                    nc.sync.dma_start(ar[:], cc_out[k * 128:(k + 1) * 128, :])
                    nc.vector.tensor_add(xt[k][:], xt[k][:], ar[:])

        def dump_x():
            for k in range(KT):
                pr = evp.tile([128, S], F32, tag="dbg")
                nc.vector.tensor_copy(pr[:], xt[k][:])
                nc.sync.dma_start(y_out[k * 128:(k + 1) * 128, :], pr[:])

        stages = []
        for l in range(L):
            stages.append((f"attn{l}", attn_block, l))
            stages.append((f"mlp{l}", mlp_block, l))
        for _ in range(reps):
            load_x()
            for sname, fn, l in stages:
                if fn is mlp_block:
                    fn(l, last=(l == L - 1 and debug_stage is None))
                else:
                    fn(l)
                    if debug_stage == f"stk{l}":
                        break
                if debug_stage == sname:
                    dump_x()
                    break

    nc.compile()
    return nc


def make_inputs(input_ids, embed, wq, wk, wv, wo, wgate, wup, wdown, ln1, ln2):
    """host-side prep: embedding gather, shard + fold norm gains/scale into weights."""
    f32 = np.float32
    bf = ml_dtypes.bfloat16
    x = np.asarray(embed, f32)[np.asarray(input_ids)[0]]      # (S, DM)
    xt = np.ascontiguousarray(x.T)                            # (DM, S)

    inv_freq = 1.0 / (THETA ** (np.arange(0, D, 2, dtype=f32) / D))
    freqs = np.arange(S, dtype=f32)[:, None] * inv_freq[None, :]    # (S, 32)
    emb = np.concatenate([freqs, freqs], axis=1)                    # (S, D)
    cosT = np.cos(emb).T.astype(f32)                                # (D, S)
    sinT = np.sin(emb).T.astype(f32)
    sinT_signed = sinT.copy()
    sinT_signed[: D // 2] *= -1.0
    cos_rep = np.concatenate([cosT, cosT], axis=0)                  # (128, S) f32
    sin_rep = np.concatenate([sinT_signed, sinT_signed], axis=0)

    mask = np.zeros((2, 128, 256), f32)
    jj = np.arange(128)[:, None]
    ii = np.arange(256)[None, :]
    for o in range(2):
        mask[o] = np.where(128 * o + jj <= ii, 0.0, NEG)

    scale_q = 1.0 / math.sqrt(D)
    in_maps = []
    for c in range(NCORES):
        wqkv = np.empty((L, DM, 384), f32)
        wo_c = np.empty((L, QC, DM), f32)
        wg_c = np.empty((L, DM, FFS), f32)
        wu_c = np.empty((L, DM, FFS), f32)
        wd_c = np.empty((L, FFS, DM), f32)
        for l in range(L):
            g1 = np.asarray(ln1[l], f32)[:, None]
            g2 = np.asarray(ln2[l], f32)[:, None]
            wqkv[l, :, :QC] = np.asarray(wq[l], f32)[:, c * QC:(c + 1) * QC] * g1 * scale_q
            wqkv[l, :, QC:QC + D] = np.asarray(wk[l], f32)[:, c * D:(c + 1) * D] * g1
            wqkv[l, :, QC + D:] = np.asarray(wv[l], f32)[:, c * D:(c + 1) * D] * g1
            wo_c[l] = np.asarray(wo[l], f32)[c * QC:(c + 1) * QC, :]
            wg_c[l] = np.asarray(wgate[l], f32)[:, c * FFS:(c + 1) * FFS] * g2
            wu_c[l] = np.asarray(wup[l], f32)[:, c * FFS:(c + 1) * FFS] * g2
            wd_c[l] = np.asarray(wdown[l], f32)[c * FFS:(c + 1) * FFS, :]
        in_maps.append({
            "xt": xt, "wqkv": wqkv.astype(bf),
            "wo": wo_c.astype(bf), "wg": wg_c.astype(bf), "wu": wu_c.astype(bf),
            "wd": wd_c.astype(bf), "cosr": cos_rep, "sinr": sin_rep, "mask": mask,
        })
    return in_maps


_NC_CACHE = {}


def kernel(**inputs) -> np.ndarray:
    if 1 not in _NC_CACHE:
        _NC_CACHE[1] = build(reps=1)
    nc = _NC_CACHE[1]
    in_maps = make_inputs(**inputs)
    res = run_bass_kernel_spmd(nc, in_maps, list(range(NCORES)))
    y = np.zeros((DM, S), np.float64)
    for c in range(NCORES):
        y += res.results[c]["y"].astype(np.float64)
    return np.ascontiguousarray(y.T.astype(np.float32)).reshape(B, S, DM)
